# revision 18
# baseline (speedup 1.0000x reference)
"""Multi-head attention (b=2, s=2048, d=1024, 16 heads) on 8 trn2 cores.

Sharding: core c -> batch c//4, head-group c%4 (4 heads each).
Data-parallel over batch, tensor-parallel over heads; the 4 partial
output projections per batch are summed on the host (the TP all-reduce).

Per-core program (matmuls in bf16, fp32 PSUM accumulation):
  lead-in: x chunks DMA'd in k order with fully-contiguous 4KB+ lines;
    kT (heads 0-1) and qT (n=0,1) accumulate k-streamed as chunks land.
  V   [2048,4,65] = x @ wv (+ bias), augmented with a ones column
  heads processed in pairs; per pair, query-chunk qc (512 wide), key
  chunk pair kc2:
    sT(kc)   = kT(kc-chunk).T @ qT    -> PSUM, two heads row-tiled
               concurrently (tile_position (0,0)/(64,0))
    E        = exp(0.125 * sT)        -> ACT (exp is the ~147us floor;
               scalar engine kept exp-only)
    out_aug += V_aug(kc).T @ E        -> PSUM [65,512]; row 64 = denom
  per qc: oa evacuated to SBUF immediately (frees PSUM for next qc),
    then OT = oa[0:64]/denom off the critical path.
  yT = projT.T @ OT per qc -> fp16, one 1MB contiguous DMA wave per qc
    (partial; host sums over 4 cores).
"""

import numpy as np

N_CORES = 8
P = 128
S = 2048
D = 1024
HD = 64
NH = 4        # heads per core
SCALE = HD ** -0.5
KC = S // P   # 16 key chunks
QC = 4        # query chunks
NQ = S // QC  # 512
KD = D // P   # 8 contraction chunks for d=1024

_CACHE = {}
DEBUG_TAPS = False


def build_program():
    import contextlib

    import concourse.mybir as mybir
    import concourse.tile as tile
    from concourse import bacc

    F32 = mybir.dt.float32
    F16 = mybir.dt.float16
    BF16 = mybir.dt.bfloat16
    Exp = mybir.ActivationFunctionType.Exp

    nc = bacc.Bacc("TRN2", target_bir_lowering=False, debug=False,
                   num_devices=N_CORES)

    # All dram tensors are pre-arranged host-side so every DMA line is
    # >=4KB contiguous per partition (descriptor-rate, not HBM, limited
    # the old layouts).
    xd = nc.dram_tensor("xd", [P, KD, S], BF16, kind="ExternalInput").ap()
    wqk = nc.dram_tensor("wqk", [P, KD, 512], BF16, kind="ExternalInput").ap()
    bqk = nc.dram_tensor("bqk", [P, 4], F32, kind="ExternalInput").ap()
    wv = nc.dram_tensor("wv", [P, KD, 256], BF16, kind="ExternalInput").ap()
    bvb = nc.dram_tensor("bvb", [P, 256], F32, kind="ExternalInput").ap()
    projT = nc.dram_tensor("projT", [P, 2, D], BF16,
                           kind="ExternalInput").ap()
    # y16[p, qc, m, sq] = yT[m*128+p, qc*512+sq] (fp16 partial)
    y16 = nc.dram_tensor("y16", [P, QC, KD, NQ], F16,
                         kind="ExternalOutput").ap()

    with tile.TileContext(nc) as tc:
        ctx = contextlib.ExitStack()
        with ctx:
            const = ctx.enter_context(tc.tile_pool(name="const", bufs=1))
            xt_pool = ctx.enter_context(tc.tile_pool(name="xt", bufs=1))
            qk_pool = ctx.enter_context(tc.tile_pool(name="qk", bufs=1))
            v_pool = ctx.enter_context(tc.tile_pool(name="v", bufs=1))
            ot_pool = ctx.enter_context(tc.tile_pool(name="ot", bufs=1))
            e_pool = ctx.enter_context(tc.tile_pool(name="e", bufs=6))
            rb_pool = ctx.enter_context(tc.tile_pool(name="rb", bufs=2))
            y_pool = ctx.enter_context(tc.tile_pool(name="y", bufs=2))
            # PSUM budget (8 banks): scores 2x[128,1024] = 4 (lead-in:
            # kT accumulators), attnV oa 2x[128,512] = 2 (lead-in: qT),
            # misc (v/qk-background/proj) 2x[128,512] = 2.
            ps_pool = ctx.enter_context(
                tc.tile_pool(name="ps", bufs=2, space="PSUM"))
            ps_oa = ctx.enter_context(
                tc.tile_pool(name="ps_oa", bufs=1, space="PSUM"))
            ps_misc = ctx.enter_context(
                tc.tile_pool(name="ps_misc", bufs=2, space="PSUM"))

            # PE warm-up first: dummy matmuls during the DMA lead-in
            # keep the HAM activity monitor busy so the k-streamed qk
            # matmuls run at 2.4 GHz (cold they can't keep up with the
            # x chunk arrival cadence).
            warm_sb = const.tile([P, NQ], BF16)
            nc.vector.memset(warm_sb[:], 1.0)
            wps = ps_misc.tile([P, NQ], F32, name="mps")
            for _ in range(16):
                nc.tensor.matmul(wps[:], lhsT=warm_sb[:, 0:P],
                                 rhs=warm_sb[:], start=True, stop=True)
            warm_out = const.tile([P, 1], F32)
            nc.vector.tensor_copy(warm_out[:], wps[:, 0:1])

            # ---- input DMAs ----
            # scalar queue: weights/biases.  x chunks are chained: each
            # chunk's dma_start is gated (tiny DVE copy) on the previous
            # chunk completing, so the DMA rings finish chunks in k
            # order instead of round-robining all of them to the end —
            # the k-streamed qk lead-in consumes each chunk as it lands.
            wqk_sb = const.tile([P, KD, 512], BF16)
            nc.scalar.dma_start(out=wqk_sb[:], in_=wqk)
            bqk_sb = const.tile([P, 4], F32)
            nc.scalar.dma_start(out=bqk_sb[:], in_=bqk)
            bvb_sb = const.tile([P, 4, HD], F32)
            nc.scalar.dma_start(out=bvb_sb[:], in_=bvb.rearrange(
                "p (h d) -> p h d", d=HD))
            wv_sb = const.tile([P, KD, 256], BF16)
            nc.scalar.dma_start(out=wv_sb[:], in_=wv)

            xt_sb = []
            gates = const.tile([1, KD, 2], BF16)
            for k in range(KD):
                t = xt_pool.tile([P, S], BF16, name=f"xt{k}")
                if k == 0:
                    nc.sync.dma_start(out=t[:], in_=xd[:, k, :])
                else:
                    # gate writes INTO t so the DMA trigger has a real
                    # WAW dependency on it (scheduler can't reorder)
                    nc.gpsimd.tensor_copy(t[0:1, 0:2],
                                          xt_sb[k - 1][0:1, 0:2])
                    nc.gpsimd.dma_start(out=t[:], in_=xd[:, k, :])
                xt_sb.append(t)

            # projT is only needed mid-kernel: keep it out of the early
            # HBM contention window
            projT_sb = const.tile([P, 2, D], BF16)
            nc.scalar.copy(gates[:, 0, :], xt_sb[2][0:1, 0:2])
            nc.scalar.dma_start(out=projT_sb[:], in_=projT)

            # persistent result tiles.  kT is stored one PADDED tile per
            # head (head h's 64 k-dims at its partition half, zeros in
            # the other half) so scores run as plain full-array K=128
            # matmuls: row-tiled K=64 pairs with distinct rhs streams
            # corrupt the (0,0) tile when they run concurrently.
            qt_sb = [qk_pool.tile([P, S], BF16, name=f"qk{m}")
                     for m in range(2)]
            kt_sb = [qk_pool.tile([P, S], BF16, name=f"kt{h}")
                     for h in range(4)]
            V_sb = v_pool.tile([P, KC, NH, HD + 1], BF16)
            ot_sb = [ot_pool.tile([P, S], BF16, name=f"ot{k}")
                     for k in range(2)]
            # zero the unused partition half of each padded kT tile
            for h in range(4):
                qb = HD * (h % 2)
                nc.vector.memset(kt_sb[h][HD - qb:HD - qb + HD, :], 0.0)

            ones_sb = const.tile([P, 1], F32)
            nc.vector.memset(ones_sb[:], 1.0)
            nc.vector.tensor_copy(
                V_sb[:, :, :, HD:HD + 1],
                ones_sb[:, None, None, :].broadcast_to([P, KC, NH, 1]))

            # ---- k-streamed lead-in: kT (m=2) all n, qT (m=0) n=0,1 ----
            # 6 MMs per x chunk arrival; kt lhsT shared (1 LDW + 4 MMs).
            kt_ps = [ps_pool.tile([P, 2 * NQ], F32, name="ps")
                     for _ in range(2)]
            qt_ps = [ps_oa.tile([P, NQ], F32, name=f"oa{i}")
                     for i in range(2)]
            for k in range(KD):
                st = (k == 0)
                sp = (k == KD - 1)
                for n in range(4):
                    nc.tensor.matmul(
                        kt_ps[n // 2][:, (n % 2) * NQ:(n % 2) * NQ + NQ],
                        lhsT=wqk_sb[:, k, 256:384],
                        rhs=xt_sb[k][:, n * NQ:(n + 1) * NQ],
                        start=st, stop=sp)
                for n in range(2):
                    nc.tensor.matmul(
                        qt_ps[n][:],
                        lhsT=wqk_sb[:, k, 0:P],
                        rhs=xt_sb[k][:, n * NQ:(n + 1) * NQ],
                        start=st, stop=sp)
            def evict_kt(pair, n, ps_ap):
                # split PSUM [128, 512] into the two padded kT tiles
                for i in range(2):
                    qb = HD * i
                    nc.vector.tensor_scalar_add(
                        kt_sb[2 * pair + i][qb:qb + HD,
                                            n * NQ:(n + 1) * NQ],
                        ps_ap[qb:qb + HD, :],
                        bqk_sb[qb:qb + HD, 2 + pair:3 + pair])

            # evictions in attention-consumption order: kc2=0 needs
            # kt n=0 + qt n=0 first
            evict_kt(0, 0, kt_ps[0][:, 0:NQ])
            nc.vector.tensor_scalar_add(
                qt_sb[0][:, 0:NQ], qt_ps[0][:], bqk_sb[:, 0:1])
            for n in range(1, 4):
                evict_kt(0, n,
                         kt_ps[n // 2][:, (n % 2) * NQ:(n % 2) * NQ + NQ])
            nc.vector.tensor_scalar_add(
                qt_sb[0][:, NQ:2 * NQ], qt_ps[1][:], bqk_sb[:, 0:1])

            # ---- qkT projection, one query/key chunk (background) ----
            def qk_chunk(m, n):
                ps = ps_misc.tile([P, NQ], F32, name="mps")
                for k in range(KD):
                    nc.tensor.matmul(
                        ps[:],
                        lhsT=wqk_sb[:, k, m * P:(m + 1) * P],
                        rhs=xt_sb[k][:, n * NQ:(n + 1) * NQ],
                        start=(k == 0), stop=(k == KD - 1))
                if m < 2:
                    nc.vector.tensor_scalar_add(
                        qt_sb[m][:, n * NQ:(n + 1) * NQ], ps[:],
                        bqk_sb[:, m:m + 1])
                else:
                    evict_kt(m - 2, n, ps[:])

            # ---- V (natural layout) + bias, one key chunk ----
            def v_chunk(mk):
                ps = ps_misc.tile([P, NQ], F32, name="mps")
                for k in range(KD):
                    nc.tensor.matmul(
                        ps[:, 0:256],
                        lhsT=xt_sb[k][:, mk * P:(mk + 1) * P],
                        rhs=wv_sb[:, k, :],
                        start=(k == 0), stop=(k == KD - 1))
                nc.vector.tensor_add(
                    V_sb[:, mk, :, 0:HD],
                    ps[:, 0:256].rearrange("p (h d) -> p h d", d=HD),
                    bvb_sb[:])

            # ---- output projection for one query chunk (partial) ----
            # fp16 eviction into a per-qc wave buffer, then one
            # contiguous 1MB DMA (8KB per partition line).
            def proj_chunk(n):
                last = (n == QC - 1)
                ysb = y_pool.tile([P, KD, NQ], F16, name="ysb")
                for m in range(KD):
                    ps = ps_misc.tile([P, NQ], F32, name="mps")
                    for k in range(2):
                        nc.tensor.matmul(
                            ps[:],
                            lhsT=projT_sb[:, k, m * P:(m + 1) * P],
                            rhs=ot_sb[k][:, n * NQ:(n + 1) * NQ],
                            start=(k == 0), stop=(k == 1))
                    # in the tail (last qc) the scalar engine is free:
                    # split evictions across engines and DMA per m-pair
                    # so the write-out overlaps the remaining projs
                    if last and m % 2 == 1:
                        nc.scalar.copy(ysb[:, m, :], ps[:])
                    else:
                        nc.vector.tensor_copy(ysb[:, m, :], ps[:])
                    if last and m % 2 == 1:
                        nc.sync.dma_start(out=y16[:, n, m - 1:m + 1, :],
                                          in_=ysb[:, m - 1:m + 1, :])
                if not last:
                    nc.sync.dma_start(out=y16[:, n, :, :], in_=ysb[:])

            # ---- attention, head pair (h0, h0+1) ----
            def attention_pair(h0, with_proj=False):
                qt = qt_sb[h0 // 2]
                for qc in range(QC):
                    oa = [ps_oa.tile([P, NQ], F32, name=f"oa{i}")
                          for i in range(2)]
                    for kc2 in range(KC // 2):
                        sc = [ps_pool.tile([P, 2 * NQ], F32, name="ps")
                              for _ in range(2)]
                        # scores: full-array K=128 against the padded kT
                        # (the other head's partition half is zeros)
                        for j in range(2):
                            kc = kc2 * 2 + j
                            for i in range(2):
                                nc.tensor.matmul(
                                    sc[i][:, j * NQ:(j + 1) * NQ],
                                    lhsT=kt_sb[h0 + i][:,
                                                       kc * P:(kc + 1) * P],
                                    rhs=qt[:, qc * NQ:(qc + 1) * NQ],
                                    start=True, stop=True)
                        es = []
                        for i in range(2):
                            e = e_pool.tile([P, 2 * NQ], BF16, name="e")
                            nc.scalar.activation(e[:], sc[i][:], Exp,
                                                 scale=SCALE)
                            es.append(e)
                        # attnV: i-outer so head0's MMs overlap EXP(h1)
                        for i in range(2):
                            for j in range(2):
                                kc = kc2 * 2 + j
                                nc.tensor.matmul(
                                    oa[i][0:HD + 1, :],
                                    lhsT=V_sb[:, kc, h0 + i, :],
                                    rhs=es[i][:, j * NQ:(j + 1) * NQ],
                                    start=(kc == 0), stop=(kc == KC - 1))
                    final_qc = with_proj and qc == QC - 1
                    if not final_qc:
                        for i in range(2):
                            h = h0 + i
                            # evacuate oa to SBUF at once: frees the
                            # PSUM bank for the next qc's attnV group.
                            # The denominator row goes to partition 0
                            # separately (reciprocal can't cross
                            # partitions).
                            oanum = rb_pool.tile([HD, NQ], F32,
                                                 name="oanum")
                            nc.vector.tensor_copy(oanum[:], oa[i][0:HD, :])
                            dens = rb_pool.tile([1, NQ], F32, name="dens")
                            nc.vector.tensor_copy(dens[:],
                                                  oa[i][HD:HD + 1, :])
                            recs = rb_pool.tile([1, NQ], F32, name="recs")
                            nc.vector.reciprocal_approx_fast(recs[:],
                                                             dens[:])
                            denb = rb_pool.tile([HD, NQ], F32, name="denb")
                            nc.gpsimd.partition_broadcast(denb[:], recs[:])
                            nc.vector.tensor_mul(
                                ot_sb[h // 2][HD * (h % 2):
                                              HD * (h % 2) + HD,
                                              qc * NQ:(qc + 1) * NQ],
                                oanum[:], denb[:])
                    else:
                        # tail: nothing reuses oa — multiply straight
                        # from PSUM, recip/bcast pipelined across heads
                        dn, rc, db = [], [], []
                        for i in range(2):
                            dens = rb_pool.tile([1, NQ], F32, name="dens")
                            nc.vector.tensor_copy(dens[:],
                                                  oa[i][HD:HD + 1, :])
                            dn.append(dens)
                        for i in range(2):
                            recs = rb_pool.tile([1, NQ], F32, name="recs")
                            nc.vector.reciprocal_approx_fast(recs[:],
                                                             dn[i][:])
                            rc.append(recs)
                            denb = rb_pool.tile([HD, NQ], F32, name="denb")
                            nc.gpsimd.partition_broadcast(denb[:],
                                                          recs[:])
                            db.append(denb)
                        # keep-warm dummies: the division window is
                        # longer than the HAM idle threshold; without
                        # these the projection matmuls run at 1.2 GHz
                        wps2 = ps_pool.tile([P, 2 * NQ], F32, name="ps")
                        for _ in range(10):
                            nc.tensor.matmul(wps2[:, 0:NQ],
                                             lhsT=warm_sb[:, 0:P],
                                             rhs=warm_sb[:],
                                             start=True, stop=True)
                        nc.scalar.copy(warm_out[:], wps2[:, 0:1])
                        for i in range(2):
                            h = h0 + i
                            nc.vector.tensor_mul(
                                ot_sb[h // 2][HD * (h % 2):
                                              HD * (h % 2) + HD,
                                              qc * NQ:(qc + 1) * NQ],
                                oa[i][0:HD, :], db[i][:])
                    if with_proj:
                        with tc.high_priority(offset=-1000000):
                            proj_chunk(qc)

            # lead-in for attention: V chunks 0..3 emitted eagerly
            v_chunk(0)
            v_chunk(1)
            v_chunk(2)
            v_chunk(3)
            # everything else attention reads, emitted ahead in program
            # order but at minimum priority: the scheduler runs it only
            # in PE gaps of the ACT-bound attention pipeline.
            with tc.high_priority(offset=-1000000):
                for mk in range(4, KC):
                    v_chunk(mk)
                for n in range(2, QC):
                    qk_chunk(0, n)
                for n in range(QC):
                    qk_chunk(1, n)
                    qk_chunk(3, n)
            attention_pair(0)
            attention_pair(2, with_proj=True)

            if DEBUG_TAPS:
                qkdbg = nc.dram_tensor("qkdbg", [P, 4, S], BF16,
                                       kind="ExternalOutput").ap()
                vdbg = nc.dram_tensor("vdbg", [P, KC, NH, HD + 1], BF16,
                                      kind="ExternalOutput").ap()
                otdbg = nc.dram_tensor("otdbg", [P, 2, S], BF16,
                                       kind="ExternalOutput").ap()
                for m in range(2):
                    nc.gpsimd.dma_start(out=qkdbg[:, m, :], in_=qt_sb[m][:])
                for i in range(2):
                    nc.gpsimd.dma_start(
                        out=qkdbg[0:HD, 2 + i, :], in_=kt_sb[2 * i][0:HD, :])
                    nc.gpsimd.dma_start(
                        out=qkdbg[HD:P, 2 + i, :],
                        in_=kt_sb[2 * i + 1][HD:P, :])
                nc.gpsimd.dma_start(out=vdbg, in_=V_sb[:])
                for m in range(2):
                    nc.gpsimd.dma_start(out=otdbg[:, m, :], in_=ot_sb[m][:])

    nc.compile()
    return nc


def get_program():
    if "nc" not in _CACHE:
        _CACHE["nc"] = build_program()
    return _CACHE["nc"]


def _bf16(a):
    import ml_dtypes

    return np.ascontiguousarray(a, np.float32).astype(ml_dtypes.bfloat16)


def shard_inputs(x, qkv_w, qkv_b, proj_w):
    """Per-core input maps. Core c: batch c//4, head group g=c%4.

    All tensors pre-arranged so each SBUF partition's DMA line is
    contiguous in DRAM.
    """
    x = np.asarray(x, np.float32)
    qkv_w = np.asarray(qkv_w, np.float32)
    qkv_b = np.asarray(qkv_b, np.float32)
    proj_w = np.asarray(proj_w, np.float32)
    xd_b = []
    for b in range(2):
        # xd[p, k, s] = x[b, s, k*128+p]
        xd_b.append(_bf16(np.ascontiguousarray(
            x[b].T.reshape(KD, P, S).transpose(1, 0, 2))))
    in_maps = []
    for c in range(N_CORES):
        b, g = divmod(c, 4)
        r0 = g * 256
        q_w = qkv_w[r0:r0 + 256]               # [256, 1024]
        k_w = qkv_w[D + r0:D + r0 + 256]
        v_w = qkv_w[2 * D + r0:2 * D + r0 + 256]
        wqkT = np.concatenate([q_w, k_w], 0).T          # [1024, 512]
        # wqk[p, k, m] = wqkT[k*128+p, m]
        wqk_arr = _bf16(np.ascontiguousarray(
            wqkT.reshape(KD, P, 512).transpose(1, 0, 2)))
        bqk_c = np.concatenate([qkv_b[r0:r0 + 256],
                                qkv_b[D + r0:D + r0 + 256]])
        bqk = np.ascontiguousarray(bqk_c.reshape(4, P).T)   # [128, 4]
        wvT = v_w.T                            # [1024, 256]
        wv_arr = _bf16(np.ascontiguousarray(
            wvT.reshape(KD, P, 256).transpose(1, 0, 2)))
        bv = qkv_b[2 * D + r0:2 * D + r0 + 256]
        bvb = np.ascontiguousarray(
            np.broadcast_to(bv, (P, 256)))     # [128, 256]
        projT_m = proj_w[:, r0:r0 + 256].T     # [256, 1024]
        # projT_arr[p, k, m] = projT_m[k*128+p, m]
        projT_arr = _bf16(np.ascontiguousarray(
            projT_m.reshape(2, P, D).transpose(1, 0, 2)))
        in_maps.append({
            "xd": xd_b[b],
            "wqk": wqk_arr,
            "bqk": bqk,
            "wv": wv_arr,
            "bvb": bvb,
            "projT": projT_arr,
        })
    return in_maps


def unshard_output(results, proj_b):
    out = np.empty((2, S, D), np.float32)
    for b in range(2):
        acc = None
        for g in range(4):
            # y16[p, qc, m, sq] -> y[s, d] with s=qc*512+sq, d=m*128+p
            y = np.asarray(results[4 * b + g]["y16"], np.float32)
            y = y.transpose(1, 3, 2, 0).reshape(S, D)
            acc = y if acc is None else acc + y
        out[b] = acc + np.asarray(proj_b, np.float32)[None, :]
    return out


def kernel(x, qkv_w, qkv_b, proj_w, proj_b):
    from concourse.bass_utils import run_bass_kernel_spmd

    nc = get_program()
    in_maps = shard_inputs(x, qkv_w, qkv_b, proj_w)
    res = run_bass_kernel_spmd(nc, in_maps, core_ids=list(range(N_CORES)))
    return unshard_output(res.results, proj_b)


# revision 19
# speedup vs baseline: 1.0839x; 1.0839x over previous
"""Multi-head attention (b=2, s=2048, d=1024, 16 heads) on 8 trn2 cores.

Sharding: core c -> batch c//4, head-group c%4 (4 heads each).
Data-parallel over batch, tensor-parallel over heads; the 4 partial
output projections per batch are summed on the host (the TP all-reduce).

Per-core program (matmuls in bf16, fp32 PSUM accumulation):
  lead-in: x chunks DMA'd in k order with fully-contiguous 4KB+ lines;
    kT (heads 0-1) and qT (n=0,1) accumulate k-streamed as chunks land.
  V   [2048,4,65] = x @ wv (+ bias), augmented with a ones column
  heads processed in pairs; per pair, query-chunk qc (512 wide), key
  chunk pair kc2:
    sT(kc)   = kT(kc-chunk).T @ qT    -> PSUM, two heads row-tiled
               concurrently (tile_position (0,0)/(64,0))
    E        = exp(0.125 * sT)        -> ACT (exp is the ~147us floor;
               scalar engine kept exp-only)
    out_aug += V_aug(kc).T @ E        -> PSUM [65,512]; row 64 = denom
  per qc: oa evacuated to SBUF immediately (frees PSUM for next qc),
    then OT = oa[0:64]/denom off the critical path.
  yT = projT.T @ OT per qc -> fp16, one 1MB contiguous DMA wave per qc
    (partial; host sums over 4 cores).
"""

import numpy as np

N_CORES = 8
P = 128
S = 2048
D = 1024
HD = 64
NH = 4        # heads per core
SCALE = HD ** -0.5
KC = S // P   # 16 key chunks
QC = 4        # query chunks
NQ = S // QC  # 512
KD = D // P   # 8 contraction chunks for d=1024

_CACHE = {}
DEBUG_TAPS = False


def build_program():
    import contextlib

    import concourse.mybir as mybir
    import concourse.tile as tile
    from concourse import bacc

    F32 = mybir.dt.float32
    F16 = mybir.dt.float16
    BF16 = mybir.dt.bfloat16
    Exp = mybir.ActivationFunctionType.Exp

    nc = bacc.Bacc("TRN2", target_bir_lowering=False, debug=False,
                   num_devices=N_CORES)

    # All dram tensors are pre-arranged host-side so every DMA line is
    # >=4KB contiguous per partition (descriptor-rate, not HBM, limited
    # the old layouts).
    xd = nc.dram_tensor("xd", [P, KD, S], BF16, kind="ExternalInput").ap()
    wqk = nc.dram_tensor("wqk", [P, KD, 512], BF16, kind="ExternalInput").ap()
    bqk = nc.dram_tensor("bqk", [P, 4], F32, kind="ExternalInput").ap()
    wv = nc.dram_tensor("wv", [P, KD, 256], BF16, kind="ExternalInput").ap()
    bvb = nc.dram_tensor("bvb", [P, 256], F32, kind="ExternalInput").ap()
    projT = nc.dram_tensor("projT", [P, 2, D], BF16,
                           kind="ExternalInput").ap()
    # y16[p, qc, m, sq] = yT[m*128+p, qc*512+sq] (fp16 partial)
    y16 = nc.dram_tensor("y16", [P, QC, KD, NQ], F16,
                         kind="ExternalOutput").ap()

    with tile.TileContext(nc) as tc:
        ctx = contextlib.ExitStack()
        with ctx:
            const = ctx.enter_context(tc.tile_pool(name="const", bufs=1))
            xt_pool = ctx.enter_context(tc.tile_pool(name="xt", bufs=1))
            qk_pool = ctx.enter_context(tc.tile_pool(name="qk", bufs=1))
            v_pool = ctx.enter_context(tc.tile_pool(name="v", bufs=1))
            ot_pool = ctx.enter_context(tc.tile_pool(name="ot", bufs=1))
            e_pool = ctx.enter_context(tc.tile_pool(name="e", bufs=6))
            rb_pool = ctx.enter_context(tc.tile_pool(name="rb", bufs=2))
            y_pool = ctx.enter_context(tc.tile_pool(name="y", bufs=2))
            # PSUM budget (8 banks): scores 2x[128,1024] = 4 (lead-in:
            # kT accumulators), attnV oa 2x[128,512] = 2 (lead-in: qT),
            # misc (v/qk-background/proj) 2x[128,512] = 2.
            ps_pool = ctx.enter_context(
                tc.tile_pool(name="ps", bufs=2, space="PSUM"))
            ps_oa = ctx.enter_context(
                tc.tile_pool(name="ps_oa", bufs=1, space="PSUM"))
            ps_misc = ctx.enter_context(
                tc.tile_pool(name="ps_misc", bufs=2, space="PSUM"))

            # PE warm-up first: dummy matmuls during the DMA lead-in
            # keep the HAM activity monitor busy so the k-streamed qk
            # matmuls run at 2.4 GHz (cold they can't keep up with the
            # x chunk arrival cadence).
            warm_sb = const.tile([P, NQ], BF16)
            nc.vector.memset(warm_sb[:], 1.0)
            wps = ps_misc.tile([P, NQ], F32, name="mps")
            for _ in range(16):
                nc.tensor.matmul(wps[:], lhsT=warm_sb[:, 0:P],
                                 rhs=warm_sb[:], start=True, stop=True)
            warm_out = const.tile([P, 1], F32)
            nc.vector.tensor_copy(warm_out[:], wps[:, 0:1])

            # ---- input DMAs ----
            # scalar queue: weights/biases.  x chunks are chained: each
            # chunk's dma_start is gated (tiny DVE copy) on the previous
            # chunk completing, so the DMA rings finish chunks in k
            # order instead of round-robining all of them to the end —
            # the k-streamed qk lead-in consumes each chunk as it lands.
            wqk_sb = const.tile([P, KD, 512], BF16)
            nc.scalar.dma_start(out=wqk_sb[:], in_=wqk)
            bqk_sb = const.tile([P, 4], F32)
            nc.scalar.dma_start(out=bqk_sb[:], in_=bqk)
            bvb_sb = const.tile([P, 4, HD], F32)
            nc.scalar.dma_start(out=bvb_sb[:], in_=bvb.rearrange(
                "p (h d) -> p h d", d=HD))
            wv_sb = const.tile([P, KD, 256], BF16)
            nc.scalar.dma_start(out=wv_sb[:], in_=wv)

            xt_sb = []
            gates = const.tile([1, KD, 2], BF16)
            for k in range(KD):
                t = xt_pool.tile([P, S], BF16, name=f"xt{k}")
                if k < 3:
                    nc.sync.dma_start(out=t[:], in_=xd[:, k, :])
                else:
                    # window-of-3 chaining: keep ~3 transfers in flight
                    # (one alone can't saturate the DMA rings) while
                    # still finishing chunks roughly in k order.  The
                    # gate writes INTO t so the DMA trigger has a real
                    # WAW dependency on it (scheduler can't reorder).
                    nc.gpsimd.tensor_copy(t[0:1, 0:2],
                                          xt_sb[k - 3][0:1, 0:2])
                    nc.gpsimd.dma_start(out=t[:], in_=xd[:, k, :])
                xt_sb.append(t)

            # projT is only needed mid-kernel: keep it out of the early
            # HBM contention window
            projT_sb = const.tile([P, 2, D], BF16)
            nc.scalar.copy(gates[:, 0, :], xt_sb[2][0:1, 0:2])
            nc.scalar.dma_start(out=projT_sb[:], in_=projT)

            # persistent result tiles.  kT is stored one PADDED tile per
            # head (head h's 64 k-dims at its partition half, zeros in
            # the other half) so scores run as plain full-array K=128
            # matmuls: row-tiled K=64 pairs with distinct rhs streams
            # corrupt the (0,0) tile when they run concurrently.
            qt_sb = [qk_pool.tile([P, S], BF16, name=f"qk{m}")
                     for m in range(2)]
            kt_sb = [qk_pool.tile([P, S], BF16, name=f"kt{h}")
                     for h in range(4)]
            V_sb = v_pool.tile([P, KC, NH, HD + 1], BF16)
            ot_sb = [ot_pool.tile([P, S], BF16, name=f"ot{k}")
                     for k in range(2)]
            # zero the unused partition half of each padded kT tile
            for h in range(4):
                qb = HD * (h % 2)
                nc.vector.memset(kt_sb[h][HD - qb:HD - qb + HD, :], 0.0)

            ones_sb = const.tile([P, 1], F32)
            nc.vector.memset(ones_sb[:], 1.0)
            nc.vector.tensor_copy(
                V_sb[:, :, :, HD:HD + 1],
                ones_sb[:, None, None, :].broadcast_to([P, KC, NH, 1]))

            # ---- k-streamed lead-in: kT (m=2) all n, qT (m=0) n=0,1 ----
            # 6 MMs per x chunk arrival; kt lhsT shared (1 LDW + 4 MMs).
            kt_ps = [ps_pool.tile([P, 2 * NQ], F32, name="ps")
                     for _ in range(2)]
            qt_ps = [ps_oa.tile([P, NQ], F32, name=f"oa{i}")
                     for i in range(2)]
            for k in range(KD):
                st = (k == 0)
                sp = (k == KD - 1)
                for n in range(4):
                    nc.tensor.matmul(
                        kt_ps[n // 2][:, (n % 2) * NQ:(n % 2) * NQ + NQ],
                        lhsT=wqk_sb[:, k, 256:384],
                        rhs=xt_sb[k][:, n * NQ:(n + 1) * NQ],
                        start=st, stop=sp)
                for n in range(2):
                    nc.tensor.matmul(
                        qt_ps[n][:],
                        lhsT=wqk_sb[:, k, 0:P],
                        rhs=xt_sb[k][:, n * NQ:(n + 1) * NQ],
                        start=st, stop=sp)
            def evict_kt(pair, n, ps_ap):
                # split PSUM [128, 512] into the two padded kT tiles
                for i in range(2):
                    qb = HD * i
                    nc.vector.tensor_scalar_add(
                        kt_sb[2 * pair + i][qb:qb + HD,
                                            n * NQ:(n + 1) * NQ],
                        ps_ap[qb:qb + HD, :],
                        bqk_sb[qb:qb + HD, 2 + pair:3 + pair])

            # evictions in attention-consumption order: kc2=0 needs
            # kt n=0 + qt n=0 first
            evict_kt(0, 0, kt_ps[0][:, 0:NQ])
            nc.vector.tensor_scalar_add(
                qt_sb[0][:, 0:NQ], qt_ps[0][:], bqk_sb[:, 0:1])
            for n in range(1, 4):
                evict_kt(0, n,
                         kt_ps[n // 2][:, (n % 2) * NQ:(n % 2) * NQ + NQ])
            nc.vector.tensor_scalar_add(
                qt_sb[0][:, NQ:2 * NQ], qt_ps[1][:], bqk_sb[:, 0:1])

            # ---- qkT projection, one query/key chunk (background) ----
            def qk_chunk(m, n):
                ps = ps_misc.tile([P, NQ], F32, name="mps")
                for k in range(KD):
                    nc.tensor.matmul(
                        ps[:],
                        lhsT=wqk_sb[:, k, m * P:(m + 1) * P],
                        rhs=xt_sb[k][:, n * NQ:(n + 1) * NQ],
                        start=(k == 0), stop=(k == KD - 1))
                if m < 2:
                    nc.vector.tensor_scalar_add(
                        qt_sb[m][:, n * NQ:(n + 1) * NQ], ps[:],
                        bqk_sb[:, m:m + 1])
                else:
                    evict_kt(m - 2, n, ps[:])

            # ---- V (natural layout) + bias, one key chunk ----
            def v_chunk(mk):
                ps = ps_misc.tile([P, NQ], F32, name="mps")
                for k in range(KD):
                    nc.tensor.matmul(
                        ps[:, 0:256],
                        lhsT=xt_sb[k][:, mk * P:(mk + 1) * P],
                        rhs=wv_sb[:, k, :],
                        start=(k == 0), stop=(k == KD - 1))
                nc.vector.tensor_add(
                    V_sb[:, mk, :, 0:HD],
                    ps[:, 0:256].rearrange("p (h d) -> p h d", d=HD),
                    bvb_sb[:])

            # ---- output projection for one query chunk (partial) ----
            # fp16 eviction into a per-qc wave buffer, then one
            # contiguous 1MB DMA (8KB per partition line).
            def proj_chunk(n):
                last = (n == QC - 1)
                ysb = y_pool.tile([P, KD, NQ], F16, name="ysb")
                for m in range(KD):
                    ps = ps_misc.tile([P, NQ], F32, name="mps")
                    for k in range(2):
                        nc.tensor.matmul(
                            ps[:],
                            lhsT=projT_sb[:, k, m * P:(m + 1) * P],
                            rhs=ot_sb[k][:, n * NQ:(n + 1) * NQ],
                            start=(k == 0), stop=(k == 1))
                    # in the tail (last qc) the scalar engine is free:
                    # split evictions across engines and DMA per m-pair
                    # so the write-out overlaps the remaining projs
                    if last and m % 2 == 1:
                        nc.scalar.copy(ysb[:, m, :], ps[:])
                    else:
                        nc.vector.tensor_copy(ysb[:, m, :], ps[:])
                    if last and m % 2 == 1:
                        nc.sync.dma_start(out=y16[:, n, m - 1:m + 1, :],
                                          in_=ysb[:, m - 1:m + 1, :])
                if not last:
                    nc.sync.dma_start(out=y16[:, n, :, :], in_=ysb[:])

            # ---- attention, head pair (h0, h0+1) ----
            def attention_pair(h0, with_proj=False):
                qt = qt_sb[h0 // 2]
                for qc in range(QC):
                    oa = [ps_oa.tile([P, NQ], F32, name=f"oa{i}")
                          for i in range(2)]
                    for kc2 in range(KC // 2):
                        sc = [ps_pool.tile([P, 2 * NQ], F32, name="ps")
                              for _ in range(2)]
                        # scores: full-array K=128 against the padded kT
                        # (the other head's partition half is zeros)
                        for j in range(2):
                            kc = kc2 * 2 + j
                            for i in range(2):
                                nc.tensor.matmul(
                                    sc[i][:, j * NQ:(j + 1) * NQ],
                                    lhsT=kt_sb[h0 + i][:,
                                                       kc * P:(kc + 1) * P],
                                    rhs=qt[:, qc * NQ:(qc + 1) * NQ],
                                    start=True, stop=True)
                        es = []
                        for i in range(2):
                            e = e_pool.tile([P, 2 * NQ], BF16, name="e")
                            nc.scalar.activation(e[:], sc[i][:], Exp,
                                                 scale=SCALE)
                            es.append(e)
                        # attnV: i-outer so head0's MMs overlap EXP(h1)
                        for i in range(2):
                            for j in range(2):
                                kc = kc2 * 2 + j
                                nc.tensor.matmul(
                                    oa[i][0:HD + 1, :],
                                    lhsT=V_sb[:, kc, h0 + i, :],
                                    rhs=es[i][:, j * NQ:(j + 1) * NQ],
                                    start=(kc == 0), stop=(kc == KC - 1))
                    final_qc = with_proj and qc == QC - 1
                    if not final_qc:
                        for i in range(2):
                            h = h0 + i
                            # evacuate oa to SBUF at once: frees the
                            # PSUM bank for the next qc's attnV group.
                            # The denominator row goes to partition 0
                            # separately (reciprocal can't cross
                            # partitions).
                            oanum = rb_pool.tile([HD, NQ], F32,
                                                 name="oanum")
                            nc.vector.tensor_copy(oanum[:], oa[i][0:HD, :])
                            dens = rb_pool.tile([1, NQ], F32, name="dens")
                            nc.vector.tensor_copy(dens[:],
                                                  oa[i][HD:HD + 1, :])
                            recs = rb_pool.tile([1, NQ], F32, name="recs")
                            nc.vector.reciprocal_approx_fast(recs[:],
                                                             dens[:])
                            denb = rb_pool.tile([HD, NQ], F32, name="denb")
                            nc.gpsimd.partition_broadcast(denb[:], recs[:])
                            nc.vector.tensor_mul(
                                ot_sb[h // 2][HD * (h % 2):
                                              HD * (h % 2) + HD,
                                              qc * NQ:(qc + 1) * NQ],
                                oanum[:], denb[:])
                    else:
                        # tail: nothing reuses oa — multiply straight
                        # from PSUM, recip/bcast pipelined across heads
                        dn, rc, db = [], [], []
                        for i in range(2):
                            dens = rb_pool.tile([1, NQ], F32, name="dens")
                            nc.vector.tensor_copy(dens[:],
                                                  oa[i][HD:HD + 1, :])
                            dn.append(dens)
                        for i in range(2):
                            recs = rb_pool.tile([1, NQ], F32, name="recs")
                            nc.vector.reciprocal_approx_fast(recs[:],
                                                             dn[i][:])
                            rc.append(recs)
                            denb = rb_pool.tile([HD, NQ], F32, name="denb")
                            nc.gpsimd.partition_broadcast(denb[:],
                                                          recs[:])
                            db.append(denb)
                        # keep-warm dummies: the division window is
                        # longer than the HAM idle threshold; without
                        # these the projection matmuls run at 1.2 GHz
                        wps2 = ps_pool.tile([P, 2 * NQ], F32, name="ps")
                        for _ in range(10):
                            nc.tensor.matmul(wps2[:, 0:NQ],
                                             lhsT=warm_sb[:, 0:P],
                                             rhs=warm_sb[:],
                                             start=True, stop=True)
                        nc.scalar.copy(warm_out[:], wps2[:, 0:1])
                        for i in range(2):
                            h = h0 + i
                            nc.vector.tensor_mul(
                                ot_sb[h // 2][HD * (h % 2):
                                              HD * (h % 2) + HD,
                                              qc * NQ:(qc + 1) * NQ],
                                oa[i][0:HD, :], db[i][:])
                    if with_proj:
                        with tc.high_priority(offset=-1000000):
                            proj_chunk(qc)

            # lead-in for attention: V chunks 0..3 emitted eagerly
            v_chunk(0)
            v_chunk(1)
            v_chunk(2)
            v_chunk(3)
            # everything else attention reads, emitted ahead in program
            # order but at minimum priority: the scheduler runs it only
            # in PE gaps of the ACT-bound attention pipeline.
            with tc.high_priority(offset=-1000000):
                for mk in range(4, KC):
                    v_chunk(mk)
                for n in range(2, QC):
                    qk_chunk(0, n)
                for n in range(QC):
                    qk_chunk(1, n)
                    qk_chunk(3, n)
            attention_pair(0)
            attention_pair(2, with_proj=True)

            if DEBUG_TAPS:
                qkdbg = nc.dram_tensor("qkdbg", [P, 4, S], BF16,
                                       kind="ExternalOutput").ap()
                vdbg = nc.dram_tensor("vdbg", [P, KC, NH, HD + 1], BF16,
                                      kind="ExternalOutput").ap()
                otdbg = nc.dram_tensor("otdbg", [P, 2, S], BF16,
                                       kind="ExternalOutput").ap()
                for m in range(2):
                    nc.gpsimd.dma_start(out=qkdbg[:, m, :], in_=qt_sb[m][:])
                for i in range(2):
                    nc.gpsimd.dma_start(
                        out=qkdbg[0:HD, 2 + i, :], in_=kt_sb[2 * i][0:HD, :])
                    nc.gpsimd.dma_start(
                        out=qkdbg[HD:P, 2 + i, :],
                        in_=kt_sb[2 * i + 1][HD:P, :])
                nc.gpsimd.dma_start(out=vdbg, in_=V_sb[:])
                for m in range(2):
                    nc.gpsimd.dma_start(out=otdbg[:, m, :], in_=ot_sb[m][:])

    nc.compile()
    return nc


def get_program():
    if "nc" not in _CACHE:
        _CACHE["nc"] = build_program()
    return _CACHE["nc"]


def _bf16(a):
    import ml_dtypes

    return np.ascontiguousarray(a, np.float32).astype(ml_dtypes.bfloat16)


def shard_inputs(x, qkv_w, qkv_b, proj_w):
    """Per-core input maps. Core c: batch c//4, head group g=c%4.

    All tensors pre-arranged so each SBUF partition's DMA line is
    contiguous in DRAM.
    """
    x = np.asarray(x, np.float32)
    qkv_w = np.asarray(qkv_w, np.float32)
    qkv_b = np.asarray(qkv_b, np.float32)
    proj_w = np.asarray(proj_w, np.float32)
    xd_b = []
    for b in range(2):
        # xd[p, k, s] = x[b, s, k*128+p]
        xd_b.append(_bf16(np.ascontiguousarray(
            x[b].T.reshape(KD, P, S).transpose(1, 0, 2))))
    in_maps = []
    for c in range(N_CORES):
        b, g = divmod(c, 4)
        r0 = g * 256
        q_w = qkv_w[r0:r0 + 256]               # [256, 1024]
        k_w = qkv_w[D + r0:D + r0 + 256]
        v_w = qkv_w[2 * D + r0:2 * D + r0 + 256]
        wqkT = np.concatenate([q_w, k_w], 0).T          # [1024, 512]
        # wqk[p, k, m] = wqkT[k*128+p, m]
        wqk_arr = _bf16(np.ascontiguousarray(
            wqkT.reshape(KD, P, 512).transpose(1, 0, 2)))
        bqk_c = np.concatenate([qkv_b[r0:r0 + 256],
                                qkv_b[D + r0:D + r0 + 256]])
        bqk = np.ascontiguousarray(bqk_c.reshape(4, P).T)   # [128, 4]
        wvT = v_w.T                            # [1024, 256]
        wv_arr = _bf16(np.ascontiguousarray(
            wvT.reshape(KD, P, 256).transpose(1, 0, 2)))
        bv = qkv_b[2 * D + r0:2 * D + r0 + 256]
        bvb = np.ascontiguousarray(
            np.broadcast_to(bv, (P, 256)))     # [128, 256]
        projT_m = proj_w[:, r0:r0 + 256].T     # [256, 1024]
        # projT_arr[p, k, m] = projT_m[k*128+p, m]
        projT_arr = _bf16(np.ascontiguousarray(
            projT_m.reshape(2, P, D).transpose(1, 0, 2)))
        in_maps.append({
            "xd": xd_b[b],
            "wqk": wqk_arr,
            "bqk": bqk,
            "wv": wv_arr,
            "bvb": bvb,
            "projT": projT_arr,
        })
    return in_maps


def unshard_output(results, proj_b):
    out = np.empty((2, S, D), np.float32)
    for b in range(2):
        acc = None
        for g in range(4):
            # y16[p, qc, m, sq] -> y[s, d] with s=qc*512+sq, d=m*128+p
            y = np.asarray(results[4 * b + g]["y16"], np.float32)
            y = y.transpose(1, 3, 2, 0).reshape(S, D)
            acc = y if acc is None else acc + y
        out[b] = acc + np.asarray(proj_b, np.float32)[None, :]
    return out


def kernel(x, qkv_w, qkv_b, proj_w, proj_b):
    from concourse.bass_utils import run_bass_kernel_spmd

    nc = get_program()
    in_maps = shard_inputs(x, qkv_w, qkv_b, proj_w)
    res = run_bass_kernel_spmd(nc, in_maps, core_ids=list(range(N_CORES)))
    return unshard_output(res.results, proj_b)


# revision 24
# speedup vs baseline: 1.1379x; 1.0498x over previous
"""Multi-head attention (b=2, s=2048, d=1024, 16 heads) on 8 trn2 cores.

Sharding: core c -> batch c//4, head-group c%4 (4 heads each).
Data-parallel over batch, tensor-parallel over heads; the 4 partial
output projections per batch are summed on the host (the TP all-reduce).

Per-core program (matmuls in bf16, fp32 PSUM accumulation):
  lead-in: x chunks DMA'd in k order with fully-contiguous 4KB+ lines;
    kT (heads 0-1) and qT (n=0,1) accumulate k-streamed as chunks land.
  V   [2048,4,65] = x @ wv (+ bias), augmented with a ones column
  heads processed in pairs; per pair, query-chunk qc (512 wide), key
  chunk pair kc2:
    sT(kc)   = kT(kc-chunk).T @ qT    -> PSUM, two heads row-tiled
               concurrently (tile_position (0,0)/(64,0))
    E        = exp(0.125 * sT)        -> ACT (exp is the ~147us floor;
               scalar engine kept exp-only)
    out_aug += V_aug(kc).T @ E        -> PSUM [65,512]; row 64 = denom
  per qc: oa evacuated to SBUF immediately (frees PSUM for next qc),
    then OT = oa[0:64]/denom off the critical path.
  yT = projT.T @ OT per qc -> fp16, one 1MB contiguous DMA wave per qc
    (partial; host sums over 4 cores).
"""

import numpy as np

N_CORES = 8
P = 128
S = 2048
D = 1024
HD = 64
NH = 4        # heads per core
SCALE = HD ** -0.5
KC = S // P   # 16 key chunks
QC = 4        # query chunks
NQ = S // QC  # 512
KD = D // P   # 8 contraction chunks for d=1024

_CACHE = {}
DEBUG_TAPS = False


def build_program():
    import contextlib

    import concourse.mybir as mybir
    import concourse.tile as tile
    from concourse import bacc

    F32 = mybir.dt.float32
    F16 = mybir.dt.float16
    BF16 = mybir.dt.bfloat16
    Exp = mybir.ActivationFunctionType.Exp

    nc = bacc.Bacc("TRN2", target_bir_lowering=False, debug=False,
                   num_devices=N_CORES)

    # All dram tensors are pre-arranged host-side so every DMA line is
    # >=4KB contiguous per partition (descriptor-rate, not HBM, limited
    # the old layouts).
    xd = nc.dram_tensor("xd", [P, KD, S], BF16, kind="ExternalInput").ap()
    wqk = nc.dram_tensor("wqk", [P, KD, 512], BF16, kind="ExternalInput").ap()
    bqk = nc.dram_tensor("bqk", [P, 4], F32, kind="ExternalInput").ap()
    wv = nc.dram_tensor("wv", [P, KD, 256], BF16, kind="ExternalInput").ap()
    bvb = nc.dram_tensor("bvb", [P, 256], F32, kind="ExternalInput").ap()
    projT = nc.dram_tensor("projT", [P, 2, D], BF16,
                           kind="ExternalInput").ap()
    # y16[p, qc, m, sq] = yT[m*128+p, qc*512+sq] (fp16 partial)
    y16 = nc.dram_tensor("y16", [P, QC, KD, NQ], F16,
                         kind="ExternalOutput").ap()

    with tile.TileContext(nc) as tc:
        ctx = contextlib.ExitStack()
        with ctx:
            const = ctx.enter_context(tc.tile_pool(name="const", bufs=1))
            xt_pool = ctx.enter_context(tc.tile_pool(name="xt", bufs=1))
            qk_pool = ctx.enter_context(tc.tile_pool(name="qk", bufs=1))
            v_pool = ctx.enter_context(tc.tile_pool(name="v", bufs=1))
            ot_pool = ctx.enter_context(tc.tile_pool(name="ot", bufs=1))
            e_pool = ctx.enter_context(tc.tile_pool(name="e", bufs=6))
            rb_pool = ctx.enter_context(tc.tile_pool(name="rb", bufs=2))
            y_pool = ctx.enter_context(tc.tile_pool(name="y", bufs=2))
            # PSUM budget (8 banks): scores 2x[128,1024] = 4 (lead-in:
            # kT accumulators), attnV oa 2x[128,512] = 2 (lead-in: qT),
            # misc (v/qk-background/proj) 2x[128,512] = 2.
            ps_pool = ctx.enter_context(
                tc.tile_pool(name="ps", bufs=2, space="PSUM"))
            ps_oa = ctx.enter_context(
                tc.tile_pool(name="ps_oa", bufs=1, space="PSUM"))
            ps_misc = ctx.enter_context(
                tc.tile_pool(name="ps_misc", bufs=2, space="PSUM"))

            # PE warm-up first: dummy matmuls during the DMA lead-in
            # keep the HAM activity monitor busy so the k-streamed qk
            # matmuls run at 2.4 GHz (cold they can't keep up with the
            # x chunk arrival cadence).
            warm_sb = const.tile([P, NQ], BF16)
            nc.vector.memset(warm_sb[:], 1.0)
            wps = ps_misc.tile([P, NQ], F32, name="mps")
            for _ in range(16):
                nc.tensor.matmul(wps[:], lhsT=warm_sb[:, 0:P],
                                 rhs=warm_sb[:], start=True, stop=True)
            warm_out = const.tile([P, 1], F32)
            nc.vector.tensor_copy(warm_out[:], wps[:, 0:1])

            # ---- input DMAs ----
            # scalar queue: weights/biases.  x chunks are chained: each
            # chunk's dma_start is gated (tiny DVE copy) on the previous
            # chunk completing, so the DMA rings finish chunks in k
            # order instead of round-robining all of them to the end —
            # the k-streamed qk lead-in consumes each chunk as it lands.
            wqk_sb = const.tile([P, KD, 512], BF16)
            nc.scalar.dma_start(out=wqk_sb[:], in_=wqk)
            bqk_sb = const.tile([P, 4], F32)
            nc.scalar.dma_start(out=bqk_sb[:], in_=bqk)
            bvb_sb = const.tile([P, 4, HD], F32)
            nc.scalar.dma_start(out=bvb_sb[:], in_=bvb.rearrange(
                "p (h d) -> p h d", d=HD))
            wv_sb = const.tile([P, KD, 256], BF16)
            nc.scalar.dma_start(out=wv_sb[:], in_=wv)

            xt_sb = []
            gates = const.tile([1, KD, 2], BF16)
            for k in range(KD):
                t = xt_pool.tile([P, S], BF16, name=f"xt{k}")
                if k < 4:
                    nc.sync.dma_start(out=t[:], in_=xd[:, k, :])
                else:
                    # window-of-4 chaining: keep ~4 transfers in flight
                    # (one alone can't saturate the DMA rings) while
                    # still finishing chunks roughly in k order.  The
                    # gate writes INTO t so the DMA trigger has a real
                    # WAW dependency on it (scheduler can't reorder).
                    nc.gpsimd.tensor_copy(t[0:1, 0:2],
                                          xt_sb[k - 4][0:1, 0:2])
                    nc.gpsimd.dma_start(out=t[:], in_=xd[:, k, :])
                xt_sb.append(t)

            # projT is only needed mid-kernel: keep it out of the early
            # HBM contention window
            projT_sb = const.tile([P, 2, D], BF16)
            nc.scalar.copy(gates[:, 0, :], xt_sb[2][0:1, 0:2])
            nc.scalar.dma_start(out=projT_sb[:], in_=projT)

            # persistent result tiles.  kT is stored one PADDED tile per
            # head (head h's 64 k-dims at its partition half, zeros in
            # the other half) so scores run as plain full-array K=128
            # matmuls: row-tiled K=64 pairs with distinct rhs streams
            # corrupt the (0,0) tile when they run concurrently.
            qt_sb = [qk_pool.tile([P, S], BF16, name=f"qk{m}")
                     for m in range(2)]
            kt_sb = [qk_pool.tile([P, S], BF16, name=f"kt{h}")
                     for h in range(4)]
            V_sb = v_pool.tile([P, KC, NH, HD + 1], BF16)
            ot_sb = [ot_pool.tile([P, S], BF16, name=f"ot{k}")
                     for k in range(2)]
            # zero the unused partition half of each padded kT tile
            for h in range(4):
                qb = HD * (h % 2)
                nc.vector.memset(kt_sb[h][HD - qb:HD - qb + HD, :], 0.0)

            ones_sb = const.tile([P, 1], F32)
            nc.vector.memset(ones_sb[:], 1.0)
            nc.vector.tensor_copy(
                V_sb[:, :, :, HD:HD + 1],
                ones_sb[:, None, None, :].broadcast_to([P, KC, NH, 1]))

            # ---- k-streamed lead-in: kT (m=2) all n, qT (m=0) n=0,1 ----
            # 6 MMs per x chunk arrival; kt lhsT shared (1 LDW + 4 MMs).
            kt_ps = [ps_pool.tile([P, 2 * NQ], F32, name="ps")
                     for _ in range(2)]
            qt_ps = [ps_oa.tile([P, NQ], F32, name=f"oa{i}")
                     for i in range(2)]
            for k in range(KD):
                st = (k == 0)
                sp = (k == KD - 1)
                for n in range(4):
                    nc.tensor.matmul(
                        kt_ps[n // 2][:, (n % 2) * NQ:(n % 2) * NQ + NQ],
                        lhsT=wqk_sb[:, k, 256:384],
                        rhs=xt_sb[k][:, n * NQ:(n + 1) * NQ],
                        start=st, stop=sp)
                for n in range(2):
                    nc.tensor.matmul(
                        qt_ps[n][:],
                        lhsT=wqk_sb[:, k, 0:P],
                        rhs=xt_sb[k][:, n * NQ:(n + 1) * NQ],
                        start=st, stop=sp)
            def evict_kt(pair, n, ps_ap):
                # split PSUM [128, 512] into the two padded kT tiles;
                # half on the scalar engine (idle until the first EXP)
                # so the evictions don't serialize on DVE
                for i in range(2):
                    qb = HD * i
                    dst = kt_sb[2 * pair + i][qb:qb + HD,
                                              n * NQ:(n + 1) * NQ]
                    if i == 0:
                        nc.vector.tensor_scalar_add(
                            dst, ps_ap[qb:qb + HD, :],
                            bqk_sb[qb:qb + HD, 2 + pair:3 + pair])
                    else:
                        nc.scalar.add(
                            dst, ps_ap[qb:qb + HD, :],
                            bqk_sb[qb:qb + HD, 2 + pair:3 + pair])

            # evictions in attention-consumption order: kc2=0 needs
            # kt n=0 + qt n=0 first
            evict_kt(0, 0, kt_ps[0][:, 0:NQ])
            nc.vector.tensor_scalar_add(
                qt_sb[0][:, 0:NQ], qt_ps[0][:], bqk_sb[:, 0:1])
            for n in range(1, 4):
                evict_kt(0, n,
                         kt_ps[n // 2][:, (n % 2) * NQ:(n % 2) * NQ + NQ])
            nc.vector.tensor_scalar_add(
                qt_sb[0][:, NQ:2 * NQ], qt_ps[1][:], bqk_sb[:, 0:1])

            # ---- qkT projection, one query/key chunk (background) ----
            def qk_chunk(m, n):
                ps = ps_misc.tile([P, NQ], F32, name="mps")
                for k in range(KD):
                    nc.tensor.matmul(
                        ps[:],
                        lhsT=wqk_sb[:, k, m * P:(m + 1) * P],
                        rhs=xt_sb[k][:, n * NQ:(n + 1) * NQ],
                        start=(k == 0), stop=(k == KD - 1))
                if m < 2:
                    nc.vector.tensor_scalar_add(
                        qt_sb[m][:, n * NQ:(n + 1) * NQ], ps[:],
                        bqk_sb[:, m:m + 1])
                else:
                    evict_kt(m - 2, n, ps[:])

            # ---- V (natural layout) + bias, one key chunk ----
            def v_chunk(mk):
                ps = ps_misc.tile([P, NQ], F32, name="mps")
                for k in range(KD):
                    nc.tensor.matmul(
                        ps[:, 0:256],
                        lhsT=xt_sb[k][:, mk * P:(mk + 1) * P],
                        rhs=wv_sb[:, k, :],
                        start=(k == 0), stop=(k == KD - 1))
                nc.vector.tensor_add(
                    V_sb[:, mk, :, 0:HD],
                    ps[:, 0:256].rearrange("p (h d) -> p h d", d=HD),
                    bvb_sb[:])

            # ---- output projection for one query chunk (partial) ----
            # fp16 eviction into a per-qc wave buffer, then one
            # contiguous 1MB DMA (8KB per partition line).
            def proj_chunk(n):
                last = (n == QC - 1)
                ysb = y_pool.tile([P, KD, NQ], F16, name="ysb")
                for m in range(KD):
                    ps = ps_misc.tile([P, NQ], F32, name="mps")
                    for k in range(2):
                        nc.tensor.matmul(
                            ps[:],
                            lhsT=projT_sb[:, k, m * P:(m + 1) * P],
                            rhs=ot_sb[k][:, n * NQ:(n + 1) * NQ],
                            start=(k == 0), stop=(k == 1))
                    # in the tail (last qc) the scalar engine is free:
                    # split evictions across engines and DMA per m-pair
                    # so the write-out overlaps the remaining projs
                    if last and m % 2 == 1:
                        nc.scalar.copy(ysb[:, m, :], ps[:])
                    else:
                        nc.vector.tensor_copy(ysb[:, m, :], ps[:])
                    if last and m % 2 == 1:
                        nc.sync.dma_start(out=y16[:, n, m - 1:m + 1, :],
                                          in_=ysb[:, m - 1:m + 1, :])
                if not last:
                    nc.sync.dma_start(out=y16[:, n, :, :], in_=ysb[:])

            # ---- attention, head pair (h0, h0+1) ----
            def attention_pair(h0, with_proj=False):
                qt = qt_sb[h0 // 2]
                for qc in range(QC):
                    oa = [ps_oa.tile([P, NQ], F32, name=f"oa{i}")
                          for i in range(2)]
                    for kc2 in range(KC // 2):
                        sc = [ps_pool.tile([P, 2 * NQ], F32, name="ps")
                              for _ in range(2)]
                        # scores: full-array K=128 against the padded kT
                        # (the other head's partition half is zeros).
                        # High priority: the static schedule otherwise
                        # slots background work ahead of the next qc's
                        # scores at qc boundaries, starving the ACT.
                        with tc.high_priority(offset=500000):
                            for j in range(2):
                                kc = kc2 * 2 + j
                                for i in range(2):
                                    nc.tensor.matmul(
                                        sc[i][:, j * NQ:(j + 1) * NQ],
                                        lhsT=kt_sb[h0 + i][
                                            :, kc * P:(kc + 1) * P],
                                        rhs=qt[:, qc * NQ:(qc + 1) * NQ],
                                        start=True, stop=True)
                        es = []
                        for i in range(2):
                            e = e_pool.tile([P, 2 * NQ], BF16, name="e")
                            nc.scalar.activation(e[:], sc[i][:], Exp,
                                                 scale=SCALE)
                            es.append(e)
                        # attnV: i-outer so head0's MMs overlap EXP(h1)
                        for i in range(2):
                            for j in range(2):
                                kc = kc2 * 2 + j
                                nc.tensor.matmul(
                                    oa[i][0:HD + 1, :],
                                    lhsT=V_sb[:, kc, h0 + i, :],
                                    rhs=es[i][:, j * NQ:(j + 1) * NQ],
                                    start=(kc == 0), stop=(kc == KC - 1))
                    final_qc = with_proj and qc == QC - 1
                    if not final_qc:
                        for i in range(2):
                            h = h0 + i
                            # evacuate oa to SBUF at once: frees the
                            # PSUM bank for the next qc's attnV group.
                            # The denominator row goes to partition 0
                            # separately (reciprocal can't cross
                            # partitions).
                            oanum = rb_pool.tile([HD, NQ], F32,
                                                 name="oanum")
                            with tc.high_priority(offset=500000):
                                nc.vector.tensor_copy(oanum[:],
                                                      oa[i][0:HD, :])
                            dens = rb_pool.tile([1, NQ], F32, name="dens")
                            nc.vector.tensor_copy(dens[:],
                                                  oa[i][HD:HD + 1, :])
                            recs = rb_pool.tile([1, NQ], F32, name="recs")
                            nc.vector.reciprocal_approx_fast(recs[:],
                                                             dens[:])
                            denb = rb_pool.tile([HD, NQ], F32, name="denb")
                            nc.gpsimd.partition_broadcast(denb[:], recs[:])
                            nc.vector.tensor_mul(
                                ot_sb[h // 2][HD * (h % 2):
                                              HD * (h % 2) + HD,
                                              qc * NQ:(qc + 1) * NQ],
                                oanum[:], denb[:])
                    else:
                        # tail: nothing reuses oa — multiply straight
                        # from PSUM, recip/bcast pipelined across heads
                        dn, rc, db = [], [], []
                        for i in range(2):
                            dens = rb_pool.tile([1, NQ], F32, name="dens")
                            nc.vector.tensor_copy(dens[:],
                                                  oa[i][HD:HD + 1, :])
                            dn.append(dens)
                        for i in range(2):
                            recs = rb_pool.tile([1, NQ], F32, name="recs")
                            nc.vector.reciprocal_approx_fast(recs[:],
                                                             dn[i][:])
                            rc.append(recs)
                            denb = rb_pool.tile([HD, NQ], F32, name="denb")
                            nc.gpsimd.partition_broadcast(denb[:],
                                                          recs[:])
                            db.append(denb)
                        # keep-warm dummies: the division window is
                        # longer than the HAM idle threshold; without
                        # these the projection matmuls run at 1.2 GHz
                        wps2 = ps_pool.tile([P, 2 * NQ], F32, name="ps")
                        for _ in range(10):
                            nc.tensor.matmul(wps2[:, 0:NQ],
                                             lhsT=warm_sb[:, 0:P],
                                             rhs=warm_sb[:],
                                             start=True, stop=True)
                        nc.scalar.copy(warm_out[:], wps2[:, 0:1])
                        for i in range(2):
                            h = h0 + i
                            nc.vector.tensor_mul(
                                ot_sb[h // 2][HD * (h % 2):
                                              HD * (h % 2) + HD,
                                              qc * NQ:(qc + 1) * NQ],
                                oa[i][0:HD, :], db[i][:])
                    if with_proj:
                        with tc.high_priority(offset=-1000000):
                            proj_chunk(qc)

            # lead-in for attention: V chunks 0..3 emitted eagerly
            v_chunk(0)
            v_chunk(1)
            v_chunk(2)
            v_chunk(3)
            # everything else attention reads, emitted ahead in program
            # order but at minimum priority: the scheduler runs it only
            # in PE gaps of the ACT-bound attention pipeline.
            with tc.high_priority(offset=-1000000):
                for mk in range(4, KC):
                    v_chunk(mk)
                for n in range(2, QC):
                    qk_chunk(0, n)
                for n in range(QC):
                    qk_chunk(1, n)
                    qk_chunk(3, n)
            attention_pair(0)
            attention_pair(2, with_proj=True)

            if DEBUG_TAPS:
                qkdbg = nc.dram_tensor("qkdbg", [P, 4, S], BF16,
                                       kind="ExternalOutput").ap()
                vdbg = nc.dram_tensor("vdbg", [P, KC, NH, HD + 1], BF16,
                                      kind="ExternalOutput").ap()
                otdbg = nc.dram_tensor("otdbg", [P, 2, S], BF16,
                                       kind="ExternalOutput").ap()
                for m in range(2):
                    nc.gpsimd.dma_start(out=qkdbg[:, m, :], in_=qt_sb[m][:])
                for i in range(2):
                    nc.gpsimd.dma_start(
                        out=qkdbg[0:HD, 2 + i, :], in_=kt_sb[2 * i][0:HD, :])
                    nc.gpsimd.dma_start(
                        out=qkdbg[HD:P, 2 + i, :],
                        in_=kt_sb[2 * i + 1][HD:P, :])
                nc.gpsimd.dma_start(out=vdbg, in_=V_sb[:])
                for m in range(2):
                    nc.gpsimd.dma_start(out=otdbg[:, m, :], in_=ot_sb[m][:])

    nc.compile()
    return nc


def get_program():
    if "nc" not in _CACHE:
        _CACHE["nc"] = build_program()
    return _CACHE["nc"]


def _bf16(a):
    import ml_dtypes

    return np.ascontiguousarray(a, np.float32).astype(ml_dtypes.bfloat16)


def shard_inputs(x, qkv_w, qkv_b, proj_w):
    """Per-core input maps. Core c: batch c//4, head group g=c%4.

    All tensors pre-arranged so each SBUF partition's DMA line is
    contiguous in DRAM.
    """
    x = np.asarray(x, np.float32)
    qkv_w = np.asarray(qkv_w, np.float32)
    qkv_b = np.asarray(qkv_b, np.float32)
    proj_w = np.asarray(proj_w, np.float32)
    xd_b = []
    for b in range(2):
        # xd[p, k, s] = x[b, s, k*128+p]
        xd_b.append(_bf16(np.ascontiguousarray(
            x[b].T.reshape(KD, P, S).transpose(1, 0, 2))))
    in_maps = []
    for c in range(N_CORES):
        b, g = divmod(c, 4)
        r0 = g * 256
        q_w = qkv_w[r0:r0 + 256]               # [256, 1024]
        k_w = qkv_w[D + r0:D + r0 + 256]
        v_w = qkv_w[2 * D + r0:2 * D + r0 + 256]
        wqkT = np.concatenate([q_w, k_w], 0).T          # [1024, 512]
        # wqk[p, k, m] = wqkT[k*128+p, m]
        wqk_arr = _bf16(np.ascontiguousarray(
            wqkT.reshape(KD, P, 512).transpose(1, 0, 2)))
        bqk_c = np.concatenate([qkv_b[r0:r0 + 256],
                                qkv_b[D + r0:D + r0 + 256]])
        bqk = np.ascontiguousarray(bqk_c.reshape(4, P).T)   # [128, 4]
        wvT = v_w.T                            # [1024, 256]
        wv_arr = _bf16(np.ascontiguousarray(
            wvT.reshape(KD, P, 256).transpose(1, 0, 2)))
        bv = qkv_b[2 * D + r0:2 * D + r0 + 256]
        bvb = np.ascontiguousarray(
            np.broadcast_to(bv, (P, 256)))     # [128, 256]
        projT_m = proj_w[:, r0:r0 + 256].T     # [256, 1024]
        # projT_arr[p, k, m] = projT_m[k*128+p, m]
        projT_arr = _bf16(np.ascontiguousarray(
            projT_m.reshape(2, P, D).transpose(1, 0, 2)))
        in_maps.append({
            "xd": xd_b[b],
            "wqk": wqk_arr,
            "bqk": bqk,
            "wv": wv_arr,
            "bvb": bvb,
            "projT": projT_arr,
        })
    return in_maps


def unshard_output(results, proj_b):
    out = np.empty((2, S, D), np.float32)
    for b in range(2):
        acc = None
        for g in range(4):
            # y16[p, qc, m, sq] -> y[s, d] with s=qc*512+sq, d=m*128+p
            y = np.asarray(results[4 * b + g]["y16"], np.float32)
            y = y.transpose(1, 3, 2, 0).reshape(S, D)
            acc = y if acc is None else acc + y
        out[b] = acc + np.asarray(proj_b, np.float32)[None, :]
    return out


def kernel(x, qkv_w, qkv_b, proj_w, proj_b):
    from concourse.bass_utils import run_bass_kernel_spmd

    nc = get_program()
    in_maps = shard_inputs(x, qkv_w, qkv_b, proj_w)
    res = run_bass_kernel_spmd(nc, in_maps, core_ids=list(range(N_CORES)))
    return unshard_output(res.results, proj_b)
